# revision 22
# baseline (speedup 1.0000x reference)
"""ContextAttention Trainium2 kernel (8 NeuronCores), v3.

Sharding: core i handles batch b=i//2, sequence half i%2 (2048 rows of N=4096).
Activations live transposed ([C, n]) so the contraction dim is on partitions;
per-(b,h) reductions over the full N complete with a pairwise f32 AllReduce
between the two half-cores of each batch.

v3 structure (vs v1 baseline at ~285us):
  - Inputs host-packed partition-major, loaded with ~14 large DMAs in
    consumption order (each DMA_DIRECT2D costs ~0.7us serial issue time).
  - k/v loop: v stays in PSUM until Pool copies it (tensor_copy on the idle
    Pool engine); kvd/lkvd reductions use the HW-validated custom DVE op
    affine_mul_reduce (out=(ko*SCALE)*v, accum=sum) - one DVE pass instead
    of mult+reduce. (The generic stt/ttr bass ops crash the DVE ucode:
    NRT_EXEC_UNIT_UNRECOVERABLE - their uop-table rows never load.)
  - kvd/lkvd are not folded into projection weights (12 ACT passes on the
    post-collective critical path in v1); they fold into the t1 product via
    affine_mul_reduce scale slots: t1x = (qbf*kvd_col)*norm_bcast.
  - s-matmuls run right after the q loop; collective DMAs on Pool.
"""

import numpy as np
import ml_dtypes

import concourse.bass as bass
import concourse.mybir as mybir
import concourse.tile as tile
from concourse import bacc
from concourse.bass_utils import run_bass_kernel_spmd

bf16 = ml_dtypes.bfloat16
dt = mybir.dt
AF = mybir.ActivationFunctionType
OP = mybir.AluOpType

P = 128
NS = 2048          # local sequence rows per core
C = 768
H = 12
D = 64
KT = 6             # C // P     (k tiles / proj tiles)
NCH = 4            # NS // 512  (matmul free-dim chunks)
FD = 512
EPS = 1e-10
SC = 10.0          # delu parameter
SCALE = D ** -0.5  # 0.125
RG = [[0, 1], [2, 3], [4, 5], [6, 7]]

# wq slot order = consumption order: k/v interleaved, then q
MORDER = [6, 12, 7, 13, 8, 14, 9, 15, 10, 16, 11, 17, 0, 1, 2, 3, 4, 5]
SLOT = {m: i for i, m in enumerate(MORDER)}

_CACHE = {}


def _build():
    nc = bacc.Bacc("TRN2", target_bir_lowering=False, debug=False, num_devices=8)

    xT_in = nc.dram_tensor("xT", [P, KT, NS], dt.bfloat16, kind="ExternalInput").ap()
    yT_in = nc.dram_tensor("yT", [P, KT, NS], dt.bfloat16, kind="ExternalInput").ap()
    w6_in = nc.dram_tensor("w6", [P, 1, C], dt.bfloat16, kind="ExternalInput").ap()
    w12_in = nc.dram_tensor("w12", [P, 1, C], dt.bfloat16, kind="ExternalInput").ap()
    wkv1_in = nc.dram_tensor("wkv1", [P, 4, C], dt.bfloat16, kind="ExternalInput").ap()
    wkv2_in = nc.dram_tensor("wkv2", [P, 6, C], dt.bfloat16, kind="ExternalInput").ap()
    wq0_in = nc.dram_tensor("wq0", [P, KT, C], dt.bfloat16, kind="ExternalInput").ap()
    p1_in = nc.dram_tensor("p1", [P, KT, C], dt.bfloat16, kind="ExternalInput").ap()
    p2_in = nc.dram_tensor("p2", [P, KT, C], dt.bfloat16, kind="ExternalInput").ap()
    b1_in = nc.dram_tensor("b1", [P, KT], dt.float32, kind="ExternalInput").ap()
    b2_in = nc.dram_tensor("b2", [P, KT], dt.float32, kind="ExternalInput").ap()
    oh_in = nc.dram_tensor("oh", [H, C], dt.bfloat16, kind="ExternalInput").ap()
    xo_out = nc.dram_tensor("xo", [C, NS], dt.bfloat16, kind="ExternalOutput").ap()
    yo_out = nc.dram_tensor("yo", [C, NS], dt.bfloat16, kind="ExternalOutput").ap()

    with tile.TileContext(nc) as tc:
        with (
            tc.tile_pool(name="persist", bufs=1) as pp,
            tc.tile_pool(name="dram", bufs=1, space="DRAM") as dram,
        ):
            ccin = dram.tile([P, 4 * KT], dt.float32)
            ccout = dram.tile([P, 4 * KT], dt.float32)

            qbf = pp.tile([P, KT, NS], dt.bfloat16)
            p1 = pp.tile([P, KT, C], dt.bfloat16)
            p2 = pp.tile([P, KT, C], dt.bfloat16)
            red = pp.tile([P, 4 * KT], dt.float32)
            gred = pp.tile([P, 4 * KT], dt.float32)
            ksum = pp.tile([P, KT], dt.float32)
            lhsT3 = pp.tile([P, KT, H], dt.bfloat16)
            oh_sb = pp.tile([H, C], dt.bfloat16)
            b1_sb = pp.tile([P, KT], dt.float32)
            b2_sb = pp.tile([P, KT], dt.float32)
            snorm = pp.tile([H, NS], dt.float32)
            snorm_bf = pp.tile([H, NS], dt.bfloat16)

            ctx_pa = tc.tile_pool(name="phA", bufs=1)
            pa = ctx_pa.__enter__()
            xT = pa.tile([P, KT, NS], dt.bfloat16)
            yT = pa.tile([P, KT, NS], dt.bfloat16)
            wq = pa.tile([P, 18, C], dt.bfloat16)

            ctx_sc = tc.tile_pool(name="scr2", bufs=2)
            scr = ctx_sc.__enter__()
            ctx_s1 = tc.tile_pool(name="scr1", bufs=1)
            sc1 = ctx_s1.__enter__()

            # ---- input DMAs: consumption order on the sync HWDGE queue.
            nc.sync.dma_start(wq[:, 0:1, :], w6_in[:])
            for kk in range(3):
                nc.sync.dma_start(xT[:, kk:kk + 1, :], xT_in[:, kk:kk + 1, :])
            nc.sync.dma_start(yT[:, 0:1, :], yT_in[:, 0:1, :])
            for kk in range(3, KT):
                nc.sync.dma_start(xT[:, kk:kk + 1, :], xT_in[:, kk:kk + 1, :])
            nc.sync.dma_start(wq[:, 1:2, :], w12_in[:])
            nc.sync.dma_start(wq[:, 2:6, :], wkv1_in[:])
            nc.sync.dma_start(yT[:, 1:3, :], yT_in[:, 1:3, :])
            nc.sync.dma_start(wq[:, 6:12, :], wkv2_in[:])
            nc.sync.dma_start(yT[:, 3:6, :], yT_in[:, 3:6, :])
            # wq0/oh/p1/p2/b1/b2 are needed late; their DMA triggers are
            # deferred into the k/v loop (scalar queue) so they don't steal
            # HBM bandwidth from xT/yT/wkv during the startup window.

            nc.vector.memset(lhsT3[:], 0.0)

            lko = pa.tile([P, KT, NS], dt.bfloat16)

            def lk_prep(j):
                """delu(yT_j) -> lko[:, j]; ACT exp/relu + DVE min/add."""
                e2 = scr.tile([P, NS], dt.bfloat16, tag="e2", bufs=1)
                r2 = scr.tile([P, NS], dt.bfloat16, tag="r2", bufs=1)
                em2 = scr.tile([P, NS], dt.bfloat16, tag="em2", bufs=1)
                nc.scalar.activation(e2[:], yT[:, j, :], AF.Exp, scale=SC)
                nc.scalar.activation(r2[:], yT[:, j, :], AF.Relu, scale=SC)
                nc.vector.tensor_scalar_min(em2[:], e2[:], 1.0)
                nc.vector.tensor_tensor(lko[:, j, :], em2[:], r2[:], OP.add)

            lk_prep(0)

            with tc.tile_pool(name="psA", bufs=1, space="PSUM") as psA:

                def mm_tile(m, tag):
                    """qkv output m-tile -> [128, NS] psum (f32)."""
                    ps = psA.tile([P, NS], dt.float32, tag=tag)
                    s = SLOT[m]
                    for kk in range(KT):
                        for ch in range(NCH):
                            cs = slice(ch * FD, (ch + 1) * FD)
                            nc.tensor.matmul(
                                ps[:, cs],
                                wq[:, s, kk * P:(kk + 1) * P],
                                xT[:, kk, cs],
                                start=(kk == 0),
                                stop=(kk == KT - 1),
                            )
                    return ps

                # ---------- k/v loop: ksum/kvd/lkvd reductions
                # k delu via exact ACT chain: rn = relu(-10z) = -10*min(z,0),
                # em = exp(-rn) = min(exp(10z),1); both halves of
                # ksum = sum(em) + sum(r) ride the free ACT accumulators.
                # lk prep for iteration j+1 is software-pipelined into
                # iteration j (yT passes first - no psum dependency).
                for j in range(KT):
                    ps_k = mm_tile(6 + j, "pk")
                    rn = scr.tile([P, NS], dt.bfloat16, tag="rn", bufs=1)
                    em = scr.tile([P, NS], dt.bfloat16, tag="em")
                    r = scr.tile([P, NS], dt.bfloat16, tag="r")
                    nc.scalar.activation(rn[:], ps_k[:], AF.Relu, scale=-SC)
                    nc.scalar.activation(r[:], ps_k[:], AF.Relu, scale=SC,
                                         accum_out=red[:, 3 * KT + j:3 * KT + j + 1])
                    nc.scalar.activation(em[:], rn[:], AF.Exp, scale=-1.0,
                                         accum_out=red[:, j:j + 1])
                    # next iteration's lk prep AFTER the psum-freeing passes
                    # (head-of-line on ACT would delay the pk recycle)
                    if j + 1 < KT:
                        lk_prep(j + 1)
                    if j == 1:
                        nc.scalar.dma_start(wq[:, 12:18, :], wq0_in[:])
                        nc.scalar.dma_start(oh_sb[:], oh_in[:])
                    if j == 3:
                        nc.scalar.dma_start(p1[:], p1_in[:])
                        nc.scalar.dma_start(p2[:], p2_in[:])
                        nc.scalar.dma_start(b1_sb[:], b1_in[:])
                        nc.scalar.dma_start(b2_sb[:], b2_in[:])

                    ko = scr.tile([P, NS], dt.bfloat16, tag="ko", bufs=1)
                    nc.vector.tensor_tensor(ko[:], em[:], r[:], OP.add)

                    ps_v = mm_tile(12 + j, "pv")
                    # kvd_j = sum((ko*SCALE)*v) with v read straight from
                    # PSUM; one fused DVE op each (affine_mul_reduce).
                    junk = sc1.tile([P, NS], dt.bfloat16, tag="junk")
                    junk2 = junk
                    nc.vector.affine_mul_reduce(
                        out=junk[:], accum_out=red[:, KT + j:KT + j + 1],
                        in0=ko[:], in1=ps_v[:], scale=SCALE, bias=0.0)
                    nc.vector.affine_mul_reduce(
                        out=junk2[:], accum_out=red[:, 2 * KT + j:2 * KT + j + 1],
                        in0=lko[:, j, :], in1=ps_v[:], scale=SCALE, bias=0.0)

                # ------ pairwise AllReduce of [ksum | kvd | lkvd] on the
                # Pool queue; hides under the q loop.
                nc.gpsimd.dma_start(ccin[:], red[:])
                nc.gpsimd.collective_compute(
                    "AllReduce", OP.add, replica_groups=RG,
                    ins=[ccin.opt()], outs=[ccout.opt()],
                )
                nc.gpsimd.dma_start(gred[:], ccout[:])

                # ---------- q loop (collective hides under it)
                for j in range(KT):
                    ps_q = mm_tile(j, "pv" if j % 2 == 0 else "pk")
                    e = scr.tile([P, NS], dt.bfloat16, tag="e")
                    r = scr.tile([P, NS], dt.bfloat16, tag="r")
                    nc.scalar.activation(e[:], ps_q[:], AF.Exp, scale=SC)
                    nc.scalar.activation(r[:], ps_q[:], AF.Relu, scale=SC)
                    em = scr.tile([P, NS], dt.bfloat16, tag="em")
                    nc.vector.tensor_scalar_min(em[:], e[:], 1.0)
                    nc.vector.tensor_tensor(qbf[:, j, :], em[:], r[:], OP.add)
                    if j == 4:
                        # lhsT3 = block one-hot filled with ksum+EPS; gred is
                        # long since ready, so no head-of-line risk, and the
                        # s-matmuls can start the moment the q loop ends.
                        nc.vector.tensor_scalar_add(ksum[:], gred[:, 0:KT], EPS)
                        nc.vector.tensor_tensor(ksum[:], ksum[:],
                                                gred[:, 3 * KT:4 * KT], OP.add)
                        for jj in range(KT):
                            nc.vector.tensor_copy(
                                lhsT3[0:64, jj, 2 * jj:2 * jj + 1],
                                ksum[0:64, jj:jj + 1])
                            nc.vector.tensor_copy(
                                lhsT3[64:128, jj, 2 * jj + 1:2 * jj + 2],
                                ksum[64:128, jj:jj + 1])


            ctx_s1.__exit__(None, None, None)
            ctx_sc.__exit__(None, None, None)

            # ---------------- phase B: s, norm, t1x/t1y, projections
            with (
                tc.tile_pool(name="phB", bufs=2) as pb,
                tc.tile_pool(name="psS", bufs=2, space="PSUM") as psS,
                tc.tile_pool(name="psB", bufs=3, space="PSUM") as psB,
                tc.tile_pool(name="psO", bufs=3, space="PSUM") as psO,
                tc.tile_pool(name="outp", bufs=4) as outp,
            ):
                for ch in range(NCH):
                    cs = slice(ch * FD, (ch + 1) * FD)
                    ps_s = psS.tile([H, FD], dt.float32, tag="s")
                    for j in range(KT):
                        nc.tensor.matmul(
                            ps_s[:],
                            lhsT3[:, j, :],
                            qbf[:, j, cs],
                            start=(j == 0),
                            stop=(j == KT - 1),
                        )
                    nc.vector.reciprocal_approx_fast(snorm[:, cs], ps_s[:])
                    nc.scalar.copy(snorm_bf[:, cs], snorm[:, cs])

                for ch in range(NCH):
                    cs = slice(ch * FD, (ch + 1) * FD)
                    t1x = pb.tile([P, KT, FD], dt.bfloat16, tag="t1x")
                    t1y = pb.tile([P, KT, FD], dt.bfloat16, tag="t1y")
                    bcc = pb.tile([P, KT, FD], dt.bfloat16, tag="bcc")
                    for j in range(KT):
                        ps_bc = psB.tile([P, FD], dt.float32, tag="bc")
                        nc.tensor.matmul(
                            ps_bc[:],
                            oh_sb[:, j * P:(j + 1) * P],
                            snorm_bf[:, cs],
                            start=True, stop=True,
                        )
                        nc.scalar.copy(bcc[:, j, :], ps_bc[:])
                    # t1x = (qbf*kvd_col)*norm ; all six x-parts first so the
                    # x-projection matmuls can start 6 DVE ops earlier.
                    ja = pb.tile([P, 1], dt.float32, tag="ja")
                    jb = pb.tile([P, 1], dt.float32, tag="jb")
                    for j in range(KT):
                        nc.vector.affine_mul_reduce(
                            out=t1x[:, j, :], accum_out=ja[:],
                            in0=qbf[:, j, cs], in1=bcc[:, j, :],
                            scale=gred[:, KT + j:KT + j + 1], bias=0.0)
                    for j in range(KT):
                        nc.vector.affine_mul_reduce(
                            out=t1y[:, j, :], accum_out=jb[:],
                            in0=qbf[:, j, cs], in1=bcc[:, j, :],
                            scale=gred[:, 2 * KT + j:2 * KT + j + 1], bias=0.0)
                    for mo in range(KT):
                        for wf, bias, t1s, dst in ((p1, b1_sb, t1x, xo_out),
                                                   (p2, b2_sb, t1y, yo_out)):
                            ps_o = psO.tile([P, FD], dt.float32, tag="o")
                            for kk in range(KT):
                                nc.tensor.matmul(
                                    ps_o[:],
                                    wf[:, kk, mo * P:(mo + 1) * P],
                                    t1s[:, kk, :],
                                    start=(kk == 0),
                                    stop=(kk == KT - 1),
                                )
                            osb = outp.tile([P, FD], dt.bfloat16, tag="osb")
                            nc.scalar.activation(osb[:], ps_o[:], AF.Identity,
                                                 bias=bias[:, mo:mo + 1],
                                                 scale=1.0)
                            nc.sync.dma_start(dst[mo * P:(mo + 1) * P, cs],
                                              osb[:])

            ctx_pa.__exit__(None, None, None)

    nc.compile()
    return nc


def _get_nc():
    if "nc" not in _CACHE:
        _CACHE["nc"] = _build()
    return _CACHE["nc"]


def _make_in_maps(x, y, qkv_w, proj1_w, proj1_b, proj2_w, proj2_b):
    wqT = np.ascontiguousarray(np.asarray(qkv_w).T).astype(bf16)  # [C, 3C]
    # [p, m, kk*128+mc] with m in slot (consumption) order
    w4 = wqT.reshape(KT, P, 18, P).transpose(1, 2, 0, 3).reshape(P, 18, C)
    ws = np.ascontiguousarray(w4[:, MORDER])
    w6 = np.ascontiguousarray(ws[:, 0:1])
    w12 = np.ascontiguousarray(ws[:, 1:2])
    wkv1 = np.ascontiguousarray(ws[:, 2:6])
    wkv2 = np.ascontiguousarray(ws[:, 6:12])
    wq0 = np.ascontiguousarray(ws[:, 12:18])

    def pmajor(wT):  # [C, C] -> [p, kk, mo*128+mc]
        a = np.asarray(wT).astype(bf16)
        return np.ascontiguousarray(a.reshape(KT, P, C).transpose(1, 0, 2))

    p1_np = pmajor(np.asarray(proj1_w).T)
    p2_np = pmajor(np.asarray(proj2_w).T)
    b1_np = np.ascontiguousarray(np.asarray(proj1_b, np.float32).reshape(KT, P).T)
    b2_np = np.ascontiguousarray(np.asarray(proj2_b, np.float32).reshape(KT, P).T)
    oh_np = np.zeros((H, C), bf16)
    for j in range(KT):
        oh_np[2 * j, j * P:j * P + 64] = 1
        oh_np[2 * j + 1, j * P + 64:(j + 1) * P] = 1
    in_maps = []
    for core in range(8):
        b_, h_ = core // 2, core % 2
        sl = slice(h_ * NS, (h_ + 1) * NS)
        xT = np.asarray(x)[b_, sl].T.astype(bf16)      # [C, NS]
        yT = np.asarray(y)[b_, sl].T.astype(bf16)
        xTp = np.ascontiguousarray(xT.reshape(KT, P, NS).transpose(1, 0, 2))
        yTp = np.ascontiguousarray(yT.reshape(KT, P, NS).transpose(1, 0, 2))
        in_maps.append({"xT": xTp, "yT": yTp, "w6": w6, "w12": w12,
                        "wkv1": wkv1, "wkv2": wkv2, "wq0": wq0,
                        "p1": p1_np, "p2": p2_np, "b1": b1_np, "b2": b2_np,
                        "oh": oh_np})
    return in_maps


def _unshard(results, B, N):
    xo = np.empty((B, N, C), np.float32)
    yo = np.empty((B, N, C), np.float32)
    for core in range(8):
        b_, h_ = core // 2, core % 2
        sl = slice(h_ * NS, (h_ + 1) * NS)
        xo[b_, sl] = results[core]["xo"].astype(np.float32).T
        yo[b_, sl] = results[core]["yo"].astype(np.float32).T
    return xo, yo


def kernel(x, y, qkv_w, proj1_w, proj1_b, proj2_w, proj2_b):
    nc = _get_nc()
    in_maps = _make_in_maps(x, y, qkv_w, proj1_w, proj1_b, proj2_w, proj2_b)
    res = run_bass_kernel_spmd(nc, in_maps, list(range(8)))
    return _unshard(res.results, np.asarray(x).shape[0], np.asarray(x).shape[1])
